# revision 33
# baseline (speedup 1.0000x reference)
"""Causal multi-head attention block (B=4, S=2048, D=768, H=12, Dh=64)
distributed over 8 NeuronCores: core = (batch, head-group), each core
computes its 6 heads end-to-end plus its partial output projection;
host sums the two partials per batch and adds the bias.

Self-contained: hardcodes all shapes; no sibling imports.
"""

import ml_dtypes
import numpy as np

B, S, D = 4, 2048, 768
H, DH = 12, 64
G = 384          # channels per head group (6 heads)
NPAIR = 3        # head pairs per core
NSC = 4          # 512-wide query windows
W = 512
NST = 16         # 128-row s-tiles
NDC = 6          # 128-row D chunks

_PROGRAM = None
PROFILE = False
PROFILE_DIR = None
LAST_RESULT = None


def _split_waits(nc, max_waits=1, max_updates=1):
    """This container's walrus rejects instructions carrying more than one
    semaphore wait/update ("Too many sync wait commands").  Move excess
    waits onto NoOps inserted before the owning instruction (same engine)
    and excess updates onto NoOps inserted after."""
    import concourse.mybir as mybir

    counter = [0]

    def nop(engine, waits, updates):
        counter[0] += 1
        n = mybir.InstNoOp(name=f"wsplit_nop_{counter[0]}", ins=[], outs=[])
        n.engine = engine
        n.sync_info = mybir.SyncInfo(on_wait=waits, on_update=updates)
        return n

    for bb in nc.main_func.blocks:
        out = []
        changed = False
        for ins in bb.instructions:
            si = ins.sync_info
            waits = list(si.on_wait) if si and si.on_wait else []
            updates = list(si.on_update) if si and si.on_update else []
            pre, post = [], []
            if len(waits) > max_waits:
                keep = waits[:max_waits - 1] if max_waits > 1 else []
                rest = waits[len(keep):]
                while rest:
                    chunk, rest = rest[:max_waits], rest[max_waits:]
                    pre.append(chunk)
                waits = keep
                changed = True
            if len(updates) > max_updates:
                rest = updates[max_updates:]
                updates = updates[:max_updates]
                while rest:
                    chunk, rest = rest[:max_updates], rest[max_updates:]
                    post.append(chunk)
                changed = True
            if pre or post:
                ins.sync_info = mybir.SyncInfo(
                    on_wait=waits, on_update=updates)
            for w in pre:
                out.append(nop(ins.engine, w, []))
            out.append(ins)
            for u in post:
                out.append(nop(ins.engine, [], u))
        if changed:
            bb.instructions = out


def _install_profile_hooks():
    """Dev-only (PROFILE=True): register the NTFF profile hook that the
    agent image's antenv lacks, and stub out the artifact upload."""
    import sys
    import types

    try:
        from antenv.axon_hooks import get_axon_ntff_profile_hook  # noqa: F401
    except ImportError:
        import antenv
        from trn_agent_boot import trn_boot

        hook = trn_boot._ntff_profile_via_ctypes("/opt/axon/libaxon_pjrt.so")
        mod = types.ModuleType("antenv.axon_hooks")
        mod._hook = hook
        mod.get_axon_ntff_profile_hook = lambda: mod._hook
        mod.set_axon_ntff_profile_hook = lambda h: setattr(mod, "_hook", h)
        sys.modules["antenv.axon_hooks"] = mod
        antenv.axon_hooks = mod

    from concourse import bass_utils

    bass_utils.upload_artifacts = lambda tmpdir: "local://" + tmpdir


def _build_program():
    import concourse.bass as bass
    import concourse.mybir as mybir
    import concourse.tile as tile

    f16 = mybir.dt.float16
    f32 = mybir.dt.float32
    f8 = mybir.dt.float8e4

    nc = bass.Bass()
    xt_d = nc.declare_dram_parameter("xt", [128, NST, NDC, 128], f16, isOutput=False)
    xt8_d = nc.declare_dram_parameter("xt8", [128, NSC, 3, 2, W], f8, isOutput=False)
    wq0_d = nc.declare_dram_parameter("wq0", [128, 3, 2, 128], f8, isOutput=False)
    wk0_d = nc.declare_dram_parameter("wk0", [128, 3, 2, 128], f8, isOutput=False)
    wq12_d = nc.declare_dram_parameter("wq12", [128, 3, 2, 256], f8, isOutput=False)
    wk12_d = nc.declare_dram_parameter("wk12", [128, 3, 2, 256], f8, isOutput=False)
    wv_d = nc.declare_dram_parameter("wv", [128, NDC, G], f16, isOutput=False)
    wo_d = nc.declare_dram_parameter("wo", [128, 3, D], f16, isOutput=False)
    mk_d = nc.declare_dram_parameter("mk", [128, 128], f16, isOutput=False)
    y_d = nc.declare_dram_parameter("y", [S, D], f16, isOutput=True)

    with tile.TileContext(nc) as tc:
        with (
            tc.tile_pool(name="const", bufs=1) as const,
            tc.tile_pool(name="work", bufs=3) as work,
            tc.tile_pool(name="outp", bufs=3) as outp,
            tc.tile_pool(name="ps", bufs=2, space="PSUM") as ps,
        ):
            # ---- persistent SBUF tiles ----
            # consolidated [128, chunk, cols] layouts: one DMA per tensor
            # (or per xt column-window) -- each dma_start trigger costs
            # ~600ns on its issuing engine and ~us-scale queue overhead,
            # so fewer+bigger transfers shorten the startup critically.
            xtb = const.tile([128, NST, NDC, 128], f16, name="xtb", tag="xtb")
            xt8b = const.tile([128, NSC, 3, 2, W], f8, name="xt8b", tag="xt8b")
            wq0b = const.tile([128, 3, 2, 128], f8, name="wq0b", tag="wq0b")
            wk0b = const.tile([128, 3, 2, 128], f8, name="wk0b", tag="wk0b")
            wq12b = const.tile([128, 3, 2, 256], f8, name="wq12b", tag="wq12b")
            wk12b = const.tile([128, 3, 2, 256], f8, name="wk12b", tag="wk12b")
            wvb = const.tile([128, NDC, G], f16, name="wvb", tag="wvb")
            wob = const.tile([128, 3, D], f16, name="wob", tag="wob")

            def xv(dc, st):  # [128, 128] x slice: key block st, d-chunk dc
                return xtb[:, st, dc, :]
            wv = [wvb[:, i, :] for i in range(NDC)]
            wo = [wob[:, i, :] for i in range(3)]

            def wqs(c, pair):  # [128, 2, 128] DoubleRow Wq slice
                if pair == 0:
                    return wq0b[:, c, :, :]
                return wq12b[:, c, :, 128 * (pair - 1):128 * pair]

            def wks(c, pair):
                if pair == 0:
                    return wk0b[:, c, :, :]
                return wk12b[:, c, :, 128 * (pair - 1):128 * pair]

            def x8s(c, sc):  # [128, 2, 512] DoubleRow x slice for window sc
                return xt8b[:, sc, c, :, :]
            # q/k stored fp8e4 in DoubleRow [part, ktile, col] layout:
            # ktile 0 carries the 64 dh values (heads at partitions 0:64 /
            # 64:128), ktile 1 is zero -- the matmul still runs at the fp8
            # double-pump rate (0.5 cyc/col), halving the scores cost.
            qt = [const.tile([128, 2, S], f8, name=f"qt{p}", tag=f"qt{p}")
                  for p in range(NPAIR)]
            kt = [const.tile([128, 2, S], f8, name=f"kt{p}", tag=f"kt{p}")
                  for p in range(NPAIR)]
            # vt slot (pair, h) = [v_h (64) | ones (64)]: the ones half makes
            # the pv matmul emit the softmax denominator (64-replicated) into
            # psum partitions 64:128 for free -- no separate dn matmuls.
            vt = [const.tile([128, NPAIR, 2, 128], f16, name=f"vt{t}",
                             tag=f"vt{t}")
                  for t in range(NST)]
            gt = [const.tile([128, S], f16, name=f"gt{p}", tag=f"gt{p}")
                  for p in range(NPAIR)]
            mk = const.tile([128, 128], f16, name="mk", tag="mk")

            # ---- input DMAs on the two HW DGE queues (gpsimd SW DGE only
            # for late-needed tensors).  Queue order matches need order:
            # scalar queue carries only the 4 weight tensors so wq0/wk0
            # land first; sync streams x windows so xt[0:4] lands by ~17us
            # (v-projections) instead of queueing behind the weights. ----
            nc.scalar.dma_start(out=wq0b, in_=wq0_d[:, :, :, :])
            nc.scalar.dma_start(out=wk0b, in_=wk0_d[:, :, :, :])
            nc.scalar.dma_start(out=wq12b, in_=wq12_d[:, :, :, :])
            nc.scalar.dma_start(out=wk12b, in_=wk12_d[:, :, :, :])
            nc.sync.dma_start(out=xt8b[:, 0, :, :, :], in_=xt8_d[:, 0, :, :, :])
            nc.sync.dma_start(out=mk, in_=mk_d[:, :])
            nc.sync.dma_start(out=xtb[:, 0:2, :, :], in_=xt_d[:, 0:2, :, :])
            nc.sync.dma_start(out=xtb[:, 2:4, :, :], in_=xt_d[:, 2:4, :, :])
            nc.sync.dma_start(out=xt8b[:, 1, :, :, :], in_=xt8_d[:, 1, :, :, :])
            nc.sync.dma_start(out=xtb[:, 4:8, :, :], in_=xt_d[:, 4:8, :, :])
            nc.sync.dma_start(out=xt8b[:, 2, :, :, :], in_=xt8_d[:, 2, :, :, :])
            nc.sync.dma_start(out=xtb[:, 8:12, :, :], in_=xt_d[:, 8:12, :, :])
            nc.gpsimd.dma_start(out=wvb, in_=wv_d[:, :, :])
            nc.gpsimd.dma_start(out=xtb[:, 12:16, :, :], in_=xt_d[:, 12:16, :, :])
            nc.gpsimd.dma_start(out=xt8b[:, 3, :, :, :], in_=xt8_d[:, 3, :, :, :])
            nc.sync.dma_start(out=wob, in_=wo_d[:, :, :])
            # memsets ordered by first use: pair-0 q/k zero halves gate the
            # first scores (~16us); vt[0:4] ones gate the first pv (~19us);
            # later vt / pairs 1-2 follow.  vt[0:4] ones go on DVE (quick,
            # lands before the first v-copy); the rest on gpsimd.
            nc.gpsimd.memset(qt[0][:, 1, :], 0.0)
            nc.gpsimd.memset(kt[0][:, 1, :], 0.0)
            for t in range(4):
                nc.vector.memset(vt[t][:, :, :, 64:128], 1.0)
            for t in range(4, NST):
                nc.gpsimd.memset(vt[t][:, :, :, 64:128], 1.0)
            for p in range(1, NPAIR):
                nc.gpsimd.memset(qt[p][:, 1, :], 0.0)
                nc.gpsimd.memset(kt[p][:, 1, :], 0.0)

            def act_copy(out, in_):
                # ScalarE Copy ('copy' is in every act table set => no
                # table swaps); keeps PSUM->SBUF casts off the DVE queue
                # where they'd sit behind window-end reciprocals.
                nc.scalar.activation(
                    out=out, in_=in_,
                    func=mybir.ActivationFunctionType.Copy, scale=1.0)

            def dve_copy(out, in_):
                nc.vector.tensor_copy(out=out, in_=in_)

            # ---- filler closures: projections and out-projections sliced
            # into single-matmul steps so they pad PE gaps between the
            # exp-paced attention matmuls without ever delaying a scores
            # matmul by more than one small filler. ----
            import collections
            NCH_PAIR = {0: 2, 1: 2, 2: 4}  # norm column chunks per pair:
            # pair 2's gt gates the out-projections, so normalize it in
            # 128-col chunks that unlock one outproj each
            GQ = collections.deque()   # (key, fn, is_last_of_key)
            done_keys = set()
            queued_keys = set()

            def push(key, fns):
                queued_keys.add(key)
                for i, f in enumerate(fns):
                    GQ.append((key, f, i == len(fns) - 1))

            def _run_one():
                k, f, is_last = GQ.popleft()
                f()
                if is_last:
                    done_keys.add(k)

            def force(key):
                # run ONLY this key's queued closures (in order), leaving
                # unrelated closures queued -- a deadline must not trigger
                # a burst of someone else's work on the PE queue
                if key not in queued_keys or key in done_keys:
                    return
                skipped = []
                while key not in done_keys:
                    k, f, is_last = GQ.popleft()
                    if k == key:
                        f()
                        if is_last:
                            done_keys.add(k)
                    else:
                        skipped.append((k, f, is_last))
                for item in reversed(skipped):
                    GQ.appendleft(item)

            def pop_n(n):
                for _ in range(n):
                    if not GQ:
                        return
                    _run_one()

            def qk_closures(pair, sc):
                # k first with a split cast: the window's first scores block
                # needs only kt[:, :128] (plus full q), so a quick 128-col
                # k-cast unblocks it ~600ns earlier; the remaining 384 k
                # columns trail behind the q cast.
                st_ = {}

                def mk(which, c):
                    wsel = wqs if which == "q" else wks
                    tgt = qt if which == "q" else kt

                    def f():
                        if c == 0:
                            st_[which] = ps.tile(
                                [128, W], f32, name=f"{which}p{pair}_{sc}",
                                tag="sc", bufs=2)
                        nc.tensor.matmul(
                            st_[which],
                            wsel(c, pair),
                            x8s(c, sc),
                            start=(c == 0), stop=(c == 2),
                            perf_mode=mybir.MatmulPerfMode.DoubleRow)
                        if c == 2 and which == "k":
                            dve_copy(
                                kt[pair][:, 0, W * sc:W * sc + 128],
                                st_["k"][:, 0:128])
                        elif c == 2:
                            dve_copy(
                                qt[pair][:, 0, W * sc:W * (sc + 1)],
                                st_["q"])
                            dve_copy(
                                kt[pair][:, 0, W * sc + 128:W * (sc + 1)],
                                st_["k"][:, 128:W])
                    return f
                return [mk(w, c) for w in ("k", "q") for c in range(3)]

            def v_closures(st):
                st_ = {}

                def mk(dc):
                    def f():
                        if dc == 0:
                            st_["vp"] = ps.tile(
                                [128, G], f32, name=f"vp{st}",
                                tag="sc", bufs=2)
                        nc.tensor.matmul(
                            st_["vp"],
                            xv(dc, st),
                            wv[dc],
                            start=(dc == 0), stop=(dc == NDC - 1))
                        if dc == NDC - 1:
                            dve_copy(
                                vt[st][:, :, :, 0:64],
                                st_["vp"].rearrange(
                                    "p (a b c) -> p a b c", a=NPAIR, b=2))
                    return f
                return [mk(dc) for dc in range(NDC)]

            def op_closures(st):
                st_ = {}

                def mk(half, cc):
                    tag = "apv" if half == 0 else "adn"
                    lo, hi = (0, G) if half == 0 else (G, D)

                    def f():
                        if half == 0 and cc == 0:
                            for p in range(NPAIR):  # gt rows for this st
                                nch = NCH_PAIR[p]
                                force(("norm", p, st // 4,
                                       (st % 4) * nch // 4))
                        if cc == 0:
                            st_[half] = ps.tile(
                                [128, G], f32, name=f"o{half}_{st}",
                                tag=tag, bufs=2)
                        nc.tensor.matmul(
                            st_[half],
                            gt[cc][:, 128 * st:128 * (st + 1)],
                            wo[cc][:, lo:hi],
                            start=(cc == 0), stop=(cc == 2))
                        if cc == 2:
                            if half == 0:
                                st_["ob"] = outp.tile(
                                    [128, D], f16, name=f"ob{st}",
                                    tag="ob", bufs=4)
                            dve_copy(st_["ob"][:, lo:hi], st_[half])
                            if half == 1:
                                nc.sync.dma_start(
                                    out=y_d[128 * st:128 * (st + 1), :],
                                    in_=st_["ob"])
                    return f
                return [mk(h, cc) for h in (0, 1) for cc in range(3)]

            class PairAttention:
                """Per-pair attention with a software pipeline that crosses
                window boundaries: pv/dn of group g are issued after the
                scores of group g+1 (even when g+1 is in the next query
                window), so neither the PE nor ACT drains at boundaries."""

                DEPTH = 3

                def __init__(self, pair):
                    self.pair = pair
                    self.NCHUNK = NCH_PAIR[pair]
                    self.pending = []

                def scores_exp(self, sc, jb):
                    pair = self.pair
                    col0 = max(0, 128 * jb - W * sc)
                    diag = jb >= 4 * sc
                    sct = ps.tile([128, 1024], f32, name=f"sc{pair}_{sc}_{jb}",
                                  tag="sc", bufs=2)
                    nc.tensor.matmul(
                        sct[:, col0:W],
                        kt[pair][0:64, :, 128 * jb:128 * (jb + 1)],
                        qt[pair][0:64, :, W * sc + col0:W * (sc + 1)],
                        start=True, stop=True,
                        perf_mode=mybir.MatmulPerfMode.DoubleRow)
                    nc.tensor.matmul(
                        sct[:, W:2 * W - col0],
                        kt[pair][64:128, :, 128 * jb:128 * (jb + 1)],
                        qt[pair][64:128, :, W * sc + col0:W * (sc + 1)],
                        start=True, stop=True,
                        perf_mode=mybir.MatmulPerfMode.DoubleRow)
                    ex = work.tile([128, 1024], f16, name=f"ex{pair}_{sc}_{jb}",
                                   tag="exp", bufs=10)
                    nc.scalar.activation(
                        out=ex[:, col0:2 * W - col0],
                        in_=sct[:, col0:2 * W - col0],
                        func=mybir.ActivationFunctionType.Exp,
                        scale=0.125 / 1024.0)
                    if diag:  # zero the j>i triangle of the diagonal block
                        # on Pool (gpsimd): keeps the jb-critical mask off the
                        # DVE queue, where the window-end reciprocal (3.3us)
                        # would delay it and starve the PE
                        nc.gpsimd.tensor_mul(
                            ex[:, col0:col0 + 128], ex[:, col0:col0 + 128], mk)
                        nc.gpsimd.tensor_mul(
                            ex[:, W:W + 128], ex[:, W:W + 128], mk)
                    return ex

                def pv_dn(self, state):
                    pair = self.pair
                    pva, pvb, sc, jb, ex = state
                    force(("v", jb))
                    col0 = max(0, 128 * jb - W * sc)
                    first, last = (jb == 0), (jb == 4 * sc + 3)
                    # [v_h | ones] stationary: rows 0:64 = attn@v, rows
                    # 64:128 = softmax denominator (64-replicated), one
                    # matmul per head instead of pv+dn pairs.
                    nc.tensor.matmul(
                        pva[:, col0:W],
                        vt[jb][:, pair, 0, :],
                        ex[:, col0:W],
                        start=first, stop=last)
                    nc.tensor.matmul(
                        pvb[:, col0:W],
                        vt[jb][:, pair, 1, :],
                        ex[:, W:2 * W - col0],
                        start=first, stop=last)
                    if last:  # window complete: drain psum into sbuf fp16
                        # with 4 quick casts (frees pva/pvb for the next
                        # window fast).  The slow reciprocal+mul then runs
                        # from sbuf as deferred filler closures, spread
                        # across the next window's DVE queue so it never
                        # delays the next window's psum-release casts.
                        pvh = work.tile([128, W], f16, name=f"ph{pair}_{sc}",
                                        tag="pvh", bufs=3)
                        dnh = work.tile([128, W], f16, name=f"dh{pair}_{sc}",
                                        tag="dnh", bufs=3)
                        rch = work.tile([128, W], f16, name=f"rh{pair}_{sc}",
                                        tag="rch", bufs=3)
                        dve_copy(pvh[0:64, :], pva[0:64, :])
                        dve_copy(dnh[0:64, :], pva[64:128, :])
                        dve_copy(pvh[64:128, :], pvb[0:64, :])
                        dve_copy(dnh[64:128, :], pvb[64:128, :])
                        nch = self.NCHUNK
                        cw = W // nch

                        def chunk(h):
                            def f():
                                a, b = cw * h, cw * (h + 1)
                                with nc.allow_low_precision(
                                        reason="softmax denom fp16 recip"):
                                    nc.vector.reciprocal(
                                        out=rch[:, a:b], in_=dnh[:, a:b])
                                nc.vector.tensor_mul(
                                    gt[pair][:, W * sc + a:W * sc + b],
                                    pvh[:, a:b], rch[:, a:b])
                            return f
                        for h in range(nch):  # per-chunk keys so outproj st
                            # forces only the 128 gt columns it reads
                            push(("norm", pair, sc, h), [chunk(h)])

                def window(self, sc, pace):
                    pair = self.pair
                    force(("qk", pair, sc))
                    pva = ps.tile([128, W], f32, name=f"pa{pair}_{sc}",
                                  tag="apv", bufs=2)
                    pvb = ps.tile([128, W], f32, name=f"pb{pair}_{sc}",
                                  tag="adn", bufs=2)
                    for jb in range(4 * sc + 4):
                        ex = self.scores_exp(sc, jb)
                        while len(self.pending) >= self.DEPTH:
                            self.pv_dn(self.pending.pop(0))
                        self.pending.append((pva, pvb, sc, jb, ex))
                        pop_n(pace)

                def flush(self):
                    while self.pending:
                        self.pv_dn(self.pending.pop(0))

            # ---- orchestration ----
            # All projection work rides the global filler queue in deadline
            # order; deadline `force`s guarantee correctness, per-jb pop_n
            # paces the queue so the exp stream is never starved by a big
            # filler burst.
            push(("qk", 0, 0), qk_closures(0, 0))
            for st in range(4):
                push(("v", st), v_closures(st))
            force(("qk", 0, 0))
            push(("qk", 0, 1), qk_closures(0, 1))
            for st in range(4, 8):
                push(("v", st), v_closures(st))
            push(("qk", 0, 2), qk_closures(0, 2))
            for st in range(8, 12):
                push(("v", st), v_closures(st))
            push(("qk", 0, 3), qk_closures(0, 3))
            for st in range(12, NST):
                push(("v", st), v_closures(st))
            for s in range(NSC):
                push(("qk", 1, s), qk_closures(1, s))
            for s in range(NSC):
                push(("qk", 2, s), qk_closures(2, s))

            pa0 = PairAttention(0)
            for sc in range(NSC):
                pa0.window(sc, pace=3)
            pa0.flush()
            pa1 = PairAttention(1)
            for sc in range(NSC):
                pa1.window(sc, pace=1)
            pa1.flush()
            pa2 = PairAttention(2)
            for sc in range(NSC):
                pa2.window(sc, pace=3)
                # flushing per pa2 window issues the normalize chain early,
                # unlocking this window's four out-projections
                pa2.flush()
                for st in range(4 * sc, 4 * sc + 4):
                    push(("op", st), op_closures(st))
            while GQ:
                pop_n(4)

    _split_waits(nc)
    return nc


def _get_program():
    global _PROGRAM
    if _PROGRAM is None:
        _PROGRAM = _build_program()
    return _PROGRAM


def kernel(x, Wq, Wk, Wv, Wo, bo):
    global LAST_RESULT
    from concourse.bass_utils import run_bass_kernel_spmd

    x = np.asarray(x, np.float32)
    Wq = np.asarray(Wq, np.float32)
    Wk = np.asarray(Wk, np.float32)
    Wv = np.asarray(Wv, np.float32)
    Wo = np.asarray(Wo, np.float32)
    bo = np.asarray(bo, np.float32)

    tri = np.tril(np.ones((128, 128), np.float32)).T  # 1 where j<=i
    mk = tri.astype(np.float16)

    in_maps = []
    for c in range(8):
        b, g = divmod(c, 2)
        hs = slice(G * g, G * (g + 1))
        def chunked(a, n):  # [n*128, M] -> [128, n, M]
            m = a.shape[1]
            return np.ascontiguousarray(
                a.reshape(n, 128, m).transpose(1, 0, 2)).astype(np.float16)

        xtw = np.ascontiguousarray(
            x[b].T.reshape(NDC, 128, NST, 128).transpose(1, 2, 0, 3)
        ).astype(np.float16)
        def pack8(a):  # [768, M] -> [128, 3, 2, M] e4m3 (d = 256c+128t+p)
            m = a.shape[1]
            return np.ascontiguousarray(
                a.reshape(3, 2, 128, m).transpose(2, 0, 1, 3)
            ).astype(ml_dtypes.float8_e4m3)

        x8 = np.ascontiguousarray(
            x[b].T.reshape(3, 2, 128, NSC, W).transpose(2, 3, 0, 1, 4)
        ).astype(ml_dtypes.float8_e4m3)
        wqt = Wq[hs, :].T * 32.0
        wkt = Wk[hs, :].T * 32.0
        in_maps.append({
            "xt": xtw,
            "xt8": x8,
            "wq0": pack8(wqt[:, 0:128]),
            "wk0": pack8(wkt[:, 0:128]),
            "wq12": pack8(wqt[:, 128:384]),
            "wk12": pack8(wkt[:, 128:384]),
            "wv": chunked(Wv[hs, :].T, NDC),
            "wo": chunked(Wo[:, hs].T, 3),
            "mk": mk,
        })

    if PROFILE:
        _install_profile_hooks()
    nc = _get_program()
    res = run_bass_kernel_spmd(nc, in_maps, core_ids=list(range(8)),
                               trace=PROFILE, tmpdir=PROFILE_DIR)
    LAST_RESULT = res
    parts = [np.asarray(res.results[c]["y"], np.float32) for c in range(8)]
    out = np.stack([parts[2 * b] + parts[2 * b + 1] + bo for b in range(B)])
    return out.astype(np.float32)



# revision 35
# speedup vs baseline: 1.0449x; 1.0449x over previous
"""Causal multi-head attention block (B=4, S=2048, D=768, H=12, Dh=64)
distributed over 8 NeuronCores: core = (batch, head-group), each core
computes its 6 heads end-to-end plus its partial output projection;
host sums the two partials per batch and adds the bias.

Self-contained: hardcodes all shapes; no sibling imports.
"""

import ml_dtypes
import numpy as np

B, S, D = 4, 2048, 768
H, DH = 12, 64
G = 384          # channels per head group (6 heads)
NPAIR = 3        # head pairs per core
NSC = 4          # 512-wide query windows
W = 512
NST = 16         # 128-row s-tiles
NDC = 6          # 128-row D chunks

_PROGRAM = None
PROFILE = False
PROFILE_DIR = None
LAST_RESULT = None


def _split_waits(nc, max_waits=1, max_updates=1):
    """This container's walrus rejects instructions carrying more than one
    semaphore wait/update ("Too many sync wait commands").  Move excess
    waits onto NoOps inserted before the owning instruction (same engine)
    and excess updates onto NoOps inserted after."""
    import concourse.mybir as mybir

    counter = [0]

    def nop(engine, waits, updates):
        counter[0] += 1
        n = mybir.InstNoOp(name=f"wsplit_nop_{counter[0]}", ins=[], outs=[])
        n.engine = engine
        n.sync_info = mybir.SyncInfo(on_wait=waits, on_update=updates)
        return n

    for bb in nc.main_func.blocks:
        out = []
        changed = False
        for ins in bb.instructions:
            si = ins.sync_info
            waits = list(si.on_wait) if si and si.on_wait else []
            updates = list(si.on_update) if si and si.on_update else []
            pre, post = [], []
            if len(waits) > max_waits:
                keep = waits[:max_waits - 1] if max_waits > 1 else []
                rest = waits[len(keep):]
                while rest:
                    chunk, rest = rest[:max_waits], rest[max_waits:]
                    pre.append(chunk)
                waits = keep
                changed = True
            if len(updates) > max_updates:
                rest = updates[max_updates:]
                updates = updates[:max_updates]
                while rest:
                    chunk, rest = rest[:max_updates], rest[max_updates:]
                    post.append(chunk)
                changed = True
            if pre or post:
                ins.sync_info = mybir.SyncInfo(
                    on_wait=waits, on_update=updates)
            for w in pre:
                out.append(nop(ins.engine, w, []))
            out.append(ins)
            for u in post:
                out.append(nop(ins.engine, [], u))
        if changed:
            bb.instructions = out


def _install_profile_hooks():
    """Dev-only (PROFILE=True): register the NTFF profile hook that the
    agent image's antenv lacks, and stub out the artifact upload."""
    import sys
    import types

    try:
        from antenv.axon_hooks import get_axon_ntff_profile_hook  # noqa: F401
    except ImportError:
        import antenv
        from trn_agent_boot import trn_boot

        hook = trn_boot._ntff_profile_via_ctypes("/opt/axon/libaxon_pjrt.so")
        mod = types.ModuleType("antenv.axon_hooks")
        mod._hook = hook
        mod.get_axon_ntff_profile_hook = lambda: mod._hook
        mod.set_axon_ntff_profile_hook = lambda h: setattr(mod, "_hook", h)
        sys.modules["antenv.axon_hooks"] = mod
        antenv.axon_hooks = mod

    from concourse import bass_utils

    bass_utils.upload_artifacts = lambda tmpdir: "local://" + tmpdir


def _build_program():
    import concourse.bass as bass
    import concourse.mybir as mybir
    import concourse.tile as tile

    f16 = mybir.dt.float16
    f32 = mybir.dt.float32
    f8 = mybir.dt.float8e4

    nc = bass.Bass()
    xt_d = nc.declare_dram_parameter("xt", [128, NST, NDC, 128], f16, isOutput=False)
    xt8_d = nc.declare_dram_parameter("xt8", [128, NSC, 3, 2, W], f8, isOutput=False)
    wq0_d = nc.declare_dram_parameter("wq0", [128, 3, 2, 128], f8, isOutput=False)
    wk0_d = nc.declare_dram_parameter("wk0", [128, 3, 2, 128], f8, isOutput=False)
    wq12_d = nc.declare_dram_parameter("wq12", [128, 3, 2, 256], f8, isOutput=False)
    wk12_d = nc.declare_dram_parameter("wk12", [128, 3, 2, 256], f8, isOutput=False)
    wv_d = nc.declare_dram_parameter("wv", [128, NDC, G], f16, isOutput=False)
    wo_d = nc.declare_dram_parameter("wo", [128, 3, D], f16, isOutput=False)
    mk_d = nc.declare_dram_parameter("mk", [128, 128], f16, isOutput=False)
    y_d = nc.declare_dram_parameter("y", [S, D], f16, isOutput=True)

    with tile.TileContext(nc) as tc:
        with (
            tc.tile_pool(name="const", bufs=1) as const,
            tc.tile_pool(name="work", bufs=3) as work,
            tc.tile_pool(name="outp", bufs=3) as outp,
            tc.tile_pool(name="ps", bufs=2, space="PSUM") as ps,
        ):
            # ---- persistent SBUF tiles ----
            # consolidated [128, chunk, cols] layouts: one DMA per tensor
            # (or per xt column-window) -- each dma_start trigger costs
            # ~600ns on its issuing engine and ~us-scale queue overhead,
            # so fewer+bigger transfers shorten the startup critically.
            xtb = const.tile([128, NST, NDC, 128], f16, name="xtb", tag="xtb")
            xt8b = const.tile([128, NSC, 3, 2, W], f8, name="xt8b", tag="xt8b")
            wq0b = const.tile([128, 3, 2, 128], f8, name="wq0b", tag="wq0b")
            wk0b = const.tile([128, 3, 2, 128], f8, name="wk0b", tag="wk0b")
            wq12b = const.tile([128, 3, 2, 256], f8, name="wq12b", tag="wq12b")
            wk12b = const.tile([128, 3, 2, 256], f8, name="wk12b", tag="wk12b")
            wvb = const.tile([128, NDC, G], f16, name="wvb", tag="wvb")
            wob = const.tile([128, 3, D], f16, name="wob", tag="wob")

            def xv(dc, st):  # [128, 128] x slice: key block st, d-chunk dc
                return xtb[:, st, dc, :]
            wv = [wvb[:, i, :] for i in range(NDC)]
            wo = [wob[:, i, :] for i in range(3)]

            def wqs(c, pair):  # [128, 2, 128] DoubleRow Wq slice
                if pair == 0:
                    return wq0b[:, c, :, :]
                return wq12b[:, c, :, 128 * (pair - 1):128 * pair]

            def wks(c, pair):
                if pair == 0:
                    return wk0b[:, c, :, :]
                return wk12b[:, c, :, 128 * (pair - 1):128 * pair]

            def x8s(c, sc):  # [128, 2, 512] DoubleRow x slice for window sc
                return xt8b[:, sc, c, :, :]
            # q/k stored fp8e4 in DoubleRow [part, ktile, col] layout:
            # ktile 0 carries the 64 dh values (heads at partitions 0:64 /
            # 64:128), ktile 1 is zero -- the matmul still runs at the fp8
            # double-pump rate (0.5 cyc/col), halving the scores cost.
            qt = [const.tile([128, 2, S], f8, name=f"qt{p}", tag=f"qt{p}")
                  for p in range(NPAIR)]
            kt = [const.tile([128, 2, S], f8, name=f"kt{p}", tag=f"kt{p}")
                  for p in range(NPAIR)]
            # vt slot (pair, h) = [v_h (64) | ones (64)]: the ones half makes
            # the pv matmul emit the softmax denominator (64-replicated) into
            # psum partitions 64:128 for free -- no separate dn matmuls.
            vt = [const.tile([128, NPAIR, 2, 128], f16, name=f"vt{t}",
                             tag=f"vt{t}")
                  for t in range(NST)]
            gt = [const.tile([128, S], f16, name=f"gt{p}", tag=f"gt{p}")
                  for p in range(NPAIR)]
            mk = const.tile([128, 128], f16, name="mk", tag="mk")

            # ---- input DMAs on the two HW DGE queues (gpsimd SW DGE only
            # for late-needed tensors).  Queue order matches need order:
            # scalar queue carries only the 4 weight tensors so wq0/wk0
            # land first; sync streams x windows so xt[0:4] lands by ~17us
            # (v-projections) instead of queueing behind the weights. ----
            nc.scalar.dma_start(out=wq0b, in_=wq0_d[:, :, :, :])
            nc.scalar.dma_start(out=wk0b, in_=wk0_d[:, :, :, :])
            nc.scalar.dma_start(out=wq12b, in_=wq12_d[:, :, :, :])
            nc.scalar.dma_start(out=wk12b, in_=wk12_d[:, :, :, :])
            nc.sync.dma_start(out=xt8b[:, 0, :, :, :], in_=xt8_d[:, 0, :, :, :])
            nc.sync.dma_start(out=mk, in_=mk_d[:, :])
            nc.sync.dma_start(out=xtb[:, 0:2, :, :], in_=xt_d[:, 0:2, :, :])
            nc.sync.dma_start(out=xtb[:, 2:4, :, :], in_=xt_d[:, 2:4, :, :])
            nc.sync.dma_start(out=xt8b[:, 1, :, :, :], in_=xt8_d[:, 1, :, :, :])
            nc.sync.dma_start(out=xtb[:, 4:8, :, :], in_=xt_d[:, 4:8, :, :])
            nc.sync.dma_start(out=xt8b[:, 2, :, :, :], in_=xt8_d[:, 2, :, :, :])
            nc.sync.dma_start(out=xtb[:, 8:12, :, :], in_=xt_d[:, 8:12, :, :])
            nc.gpsimd.dma_start(out=wvb, in_=wv_d[:, :, :])
            nc.gpsimd.dma_start(out=xtb[:, 12:16, :, :], in_=xt_d[:, 12:16, :, :])
            nc.gpsimd.dma_start(out=xt8b[:, 3, :, :, :], in_=xt8_d[:, 3, :, :, :])
            nc.sync.dma_start(out=wob, in_=wo_d[:, :, :])
            # memsets ordered by first use: pair-0 q/k zero halves gate the
            # first scores (~16us); vt[0:4] ones gate the first pv (~19us);
            # later vt / pairs 1-2 follow.  vt[0:4] ones go on DVE (quick,
            # lands before the first v-copy); the rest on gpsimd.
            nc.gpsimd.memset(qt[0][:, 1, :], 0.0)
            nc.gpsimd.memset(kt[0][:, 1, :], 0.0)
            for t in range(4):
                nc.vector.memset(vt[t][:, :, :, 64:128], 1.0)
            for t in range(4, NST):
                nc.gpsimd.memset(vt[t][:, :, :, 64:128], 1.0)
            for p in range(1, NPAIR):
                nc.gpsimd.memset(qt[p][:, 1, :], 0.0)
                nc.gpsimd.memset(kt[p][:, 1, :], 0.0)

            def act_copy(out, in_):
                # ScalarE Copy ('copy' is in every act table set => no
                # table swaps); keeps PSUM->SBUF casts off the DVE queue
                # where they'd sit behind window-end reciprocals.
                nc.scalar.activation(
                    out=out, in_=in_,
                    func=mybir.ActivationFunctionType.Copy, scale=1.0)

            def dve_copy(out, in_):
                nc.vector.tensor_copy(out=out, in_=in_)

            # ---- filler closures: projections and out-projections sliced
            # into single-matmul steps so they pad PE gaps between the
            # exp-paced attention matmuls without ever delaying a scores
            # matmul by more than one small filler. ----
            import collections
            NCH_PAIR = {0: 1, 1: 1, 2: 4}  # norm column chunks per pair:
            # pair 2's gt gates the out-projections, so normalize it in
            # 128-col chunks that unlock one outproj each
            GQ = collections.deque()   # (key, fn, is_last_of_key)
            done_keys = set()
            queued_keys = set()

            def push(key, fns):
                queued_keys.add(key)
                for i, f in enumerate(fns):
                    GQ.append((key, f, i == len(fns) - 1))

            def _run_one():
                k, f, is_last = GQ.popleft()
                f()
                if is_last:
                    done_keys.add(k)

            def force(key):
                # run ONLY this key's queued closures (in order), leaving
                # unrelated closures queued -- a deadline must not trigger
                # a burst of someone else's work on the PE queue
                if key not in queued_keys or key in done_keys:
                    return
                skipped = []
                while key not in done_keys:
                    k, f, is_last = GQ.popleft()
                    if k == key:
                        f()
                        if is_last:
                            done_keys.add(k)
                    else:
                        skipped.append((k, f, is_last))
                for item in reversed(skipped):
                    GQ.appendleft(item)

            def pop_n(n):
                for _ in range(n):
                    if not GQ:
                        return
                    _run_one()

            def qk_closures(pair, sc):
                st_ = {}

                def mk(which, c):
                    wsel = wqs if which == "q" else wks
                    tgt = qt if which == "q" else kt

                    def f():
                        if c == 0:
                            st_[which] = ps.tile(
                                [128, W], f32, name=f"{which}p{pair}_{sc}",
                                tag="sc", bufs=2)
                        nc.tensor.matmul(
                            st_[which],
                            wsel(c, pair),
                            x8s(c, sc),
                            start=(c == 0), stop=(c == 2),
                            perf_mode=mybir.MatmulPerfMode.DoubleRow)
                        if c == 2:
                            dve_copy(
                                tgt[pair][:, 0, W * sc:W * (sc + 1)],
                                st_[which])
                    return f
                return [mk(w, c) for w in ("q", "k") for c in range(3)]

            def v_closures(st):
                st_ = {}

                def mk(dc):
                    def f():
                        if dc == 0:
                            st_["vp"] = ps.tile(
                                [128, G], f32, name=f"vp{st}",
                                tag="sc", bufs=2)
                        nc.tensor.matmul(
                            st_["vp"],
                            xv(dc, st),
                            wv[dc],
                            start=(dc == 0), stop=(dc == NDC - 1))
                        if dc == NDC - 1:
                            dve_copy(
                                vt[st][:, :, :, 0:64],
                                st_["vp"].rearrange(
                                    "p (a b c) -> p a b c", a=NPAIR, b=2))
                    return f
                return [mk(dc) for dc in range(NDC)]

            def op_closures(st):
                st_ = {}

                def mk(half, cc):
                    tag = "apv" if half == 0 else "adn"
                    lo, hi = (0, G) if half == 0 else (G, D)

                    def f():
                        if half == 0 and cc == 0:
                            for p in range(NPAIR):  # gt rows for this st
                                nch = NCH_PAIR[p]
                                force(("norm", p, st // 4,
                                       (st % 4) * nch // 4))
                        if cc == 0:
                            st_[half] = ps.tile(
                                [128, G], f32, name=f"o{half}_{st}",
                                tag=tag, bufs=2)
                        nc.tensor.matmul(
                            st_[half],
                            gt[cc][:, 128 * st:128 * (st + 1)],
                            wo[cc][:, lo:hi],
                            start=(cc == 0), stop=(cc == 2))
                        if cc == 2:
                            if half == 0:
                                st_["ob"] = outp.tile(
                                    [128, D], f16, name=f"ob{st}",
                                    tag="ob", bufs=4)
                            dve_copy(st_["ob"][:, lo:hi], st_[half])
                            if half == 1:
                                nc.sync.dma_start(
                                    out=y_d[128 * st:128 * (st + 1), :],
                                    in_=st_["ob"])
                    return f
                return [mk(h, cc) for h in (0, 1) for cc in range(3)]

            class PairAttention:
                """Per-pair attention with a software pipeline that crosses
                window boundaries: pv/dn of group g are issued after the
                scores of group g+1 (even when g+1 is in the next query
                window), so neither the PE nor ACT drains at boundaries."""

                DEPTH = 3

                def __init__(self, pair):
                    self.pair = pair
                    self.NCHUNK = NCH_PAIR[pair]
                    self.pending = []

                def scores_exp(self, sc, jb):
                    pair = self.pair
                    col0 = max(0, 128 * jb - W * sc)
                    diag = jb >= 4 * sc
                    sct = ps.tile([128, 1024], f32, name=f"sc{pair}_{sc}_{jb}",
                                  tag="sc", bufs=2)
                    nc.tensor.matmul(
                        sct[:, col0:W],
                        kt[pair][0:64, :, 128 * jb:128 * (jb + 1)],
                        qt[pair][0:64, :, W * sc + col0:W * (sc + 1)],
                        start=True, stop=True,
                        perf_mode=mybir.MatmulPerfMode.DoubleRow)
                    nc.tensor.matmul(
                        sct[:, W:2 * W - col0],
                        kt[pair][64:128, :, 128 * jb:128 * (jb + 1)],
                        qt[pair][64:128, :, W * sc + col0:W * (sc + 1)],
                        start=True, stop=True,
                        perf_mode=mybir.MatmulPerfMode.DoubleRow)
                    ex = work.tile([128, 1024], f16, name=f"ex{pair}_{sc}_{jb}",
                                   tag="exp", bufs=10)
                    nc.scalar.activation(
                        out=ex[:, col0:2 * W - col0],
                        in_=sct[:, col0:2 * W - col0],
                        func=mybir.ActivationFunctionType.Exp,
                        scale=0.125 / 1024.0)
                    if diag:  # zero the j>i triangle of the diagonal block
                        # on Pool (gpsimd): keeps the jb-critical mask off the
                        # DVE queue, where the window-end reciprocal (3.3us)
                        # would delay it and starve the PE
                        nc.gpsimd.tensor_mul(
                            ex[:, col0:col0 + 128], ex[:, col0:col0 + 128], mk)
                        nc.gpsimd.tensor_mul(
                            ex[:, W:W + 128], ex[:, W:W + 128], mk)
                    return ex

                def pv_dn(self, state):
                    pair = self.pair
                    pva, pvb, sc, jb, ex = state
                    force(("v", jb))
                    col0 = max(0, 128 * jb - W * sc)
                    first, last = (jb == 0), (jb == 4 * sc + 3)
                    # [v_h | ones] stationary: rows 0:64 = attn@v, rows
                    # 64:128 = softmax denominator (64-replicated), one
                    # matmul per head instead of pv+dn pairs.
                    nc.tensor.matmul(
                        pva[:, col0:W],
                        vt[jb][:, pair, 0, :],
                        ex[:, col0:W],
                        start=first, stop=last)
                    nc.tensor.matmul(
                        pvb[:, col0:W],
                        vt[jb][:, pair, 1, :],
                        ex[:, W:2 * W - col0],
                        start=first, stop=last)
                    if last:  # window complete: drain psum into sbuf fp16
                        # with 4 quick casts (frees pva/pvb for the next
                        # window fast).  The slow reciprocal+mul then runs
                        # from sbuf as deferred filler closures, spread
                        # across the next window's DVE queue so it never
                        # delays the next window's psum-release casts.
                        pvh = work.tile([128, W], f16, name=f"ph{pair}_{sc}",
                                        tag="pvh", bufs=3)
                        dnh = work.tile([128, W], f16, name=f"dh{pair}_{sc}",
                                        tag="dnh", bufs=3)
                        rch = work.tile([128, W], f16, name=f"rh{pair}_{sc}",
                                        tag="rch", bufs=3)
                        dve_copy(pvh[0:64, :], pva[0:64, :])
                        dve_copy(dnh[0:64, :], pva[64:128, :])
                        dve_copy(pvh[64:128, :], pvb[0:64, :])
                        dve_copy(dnh[64:128, :], pvb[64:128, :])
                        nch = self.NCHUNK
                        cw = W // nch

                        def chunk(h):
                            def f():
                                a, b = cw * h, cw * (h + 1)
                                with nc.allow_low_precision(
                                        reason="softmax denom fp16 recip"):
                                    nc.vector.reciprocal(
                                        out=rch[:, a:b], in_=dnh[:, a:b])
                                nc.vector.tensor_mul(
                                    gt[pair][:, W * sc + a:W * sc + b],
                                    pvh[:, a:b], rch[:, a:b])
                            return f
                        for h in range(nch):  # per-chunk keys so outproj st
                            # forces only the 128 gt columns it reads
                            push(("norm", pair, sc, h), [chunk(h)])

                def window(self, sc, pace):
                    pair = self.pair
                    force(("qk", pair, sc))
                    pva = ps.tile([128, W], f32, name=f"pa{pair}_{sc}",
                                  tag="apv", bufs=2)
                    pvb = ps.tile([128, W], f32, name=f"pb{pair}_{sc}",
                                  tag="adn", bufs=2)
                    for jb in range(4 * sc + 4):
                        ex = self.scores_exp(sc, jb)
                        while len(self.pending) >= self.DEPTH:
                            self.pv_dn(self.pending.pop(0))
                        self.pending.append((pva, pvb, sc, jb, ex))
                        pop_n(pace)

                def flush(self):
                    while self.pending:
                        self.pv_dn(self.pending.pop(0))

            # ---- orchestration ----
            # All projection work rides the global filler queue in deadline
            # order; deadline `force`s guarantee correctness, per-jb pop_n
            # paces the queue so the exp stream is never starved by a big
            # filler burst.
            push(("qk", 0, 0), qk_closures(0, 0))
            for st in range(4):
                push(("v", st), v_closures(st))
            force(("qk", 0, 0))
            push(("qk", 0, 1), qk_closures(0, 1))
            for st in range(4, 8):
                push(("v", st), v_closures(st))
            push(("qk", 0, 2), qk_closures(0, 2))
            for st in range(8, 12):
                push(("v", st), v_closures(st))
            push(("qk", 0, 3), qk_closures(0, 3))
            for st in range(12, NST):
                push(("v", st), v_closures(st))
            for s in range(NSC):
                push(("qk", 1, s), qk_closures(1, s))
            for s in range(NSC):
                push(("qk", 2, s), qk_closures(2, s))

            pa0 = PairAttention(0)
            for sc in range(NSC):
                pa0.window(sc, pace=3)
            pa0.flush()
            pa1 = PairAttention(1)
            for sc in range(NSC):
                pa1.window(sc, pace=1)
            pa1.flush()
            pa2 = PairAttention(2)
            for sc in range(NSC):
                pa2.window(sc, pace=2)
                # flushing per pa2 window issues the normalize chain early,
                # unlocking this window's four out-projections
                pa2.flush()
                for st in range(4 * sc, 4 * sc + 4):
                    push(("op", st), op_closures(st))
            while GQ:
                pop_n(4)

    _split_waits(nc)
    return nc


def _get_program():
    global _PROGRAM
    if _PROGRAM is None:
        _PROGRAM = _build_program()
    return _PROGRAM


def kernel(x, Wq, Wk, Wv, Wo, bo):
    global LAST_RESULT
    from concourse.bass_utils import run_bass_kernel_spmd

    x = np.asarray(x, np.float32)
    Wq = np.asarray(Wq, np.float32)
    Wk = np.asarray(Wk, np.float32)
    Wv = np.asarray(Wv, np.float32)
    Wo = np.asarray(Wo, np.float32)
    bo = np.asarray(bo, np.float32)

    tri = np.tril(np.ones((128, 128), np.float32)).T  # 1 where j<=i
    mk = tri.astype(np.float16)

    in_maps = []
    for c in range(8):
        b, g = divmod(c, 2)
        hs = slice(G * g, G * (g + 1))
        def chunked(a, n):  # [n*128, M] -> [128, n, M]
            m = a.shape[1]
            return np.ascontiguousarray(
                a.reshape(n, 128, m).transpose(1, 0, 2)).astype(np.float16)

        xtw = np.ascontiguousarray(
            x[b].T.reshape(NDC, 128, NST, 128).transpose(1, 2, 0, 3)
        ).astype(np.float16)
        def pack8(a):  # [768, M] -> [128, 3, 2, M] e4m3 (d = 256c+128t+p)
            m = a.shape[1]
            return np.ascontiguousarray(
                a.reshape(3, 2, 128, m).transpose(2, 0, 1, 3)
            ).astype(ml_dtypes.float8_e4m3)

        x8 = np.ascontiguousarray(
            x[b].T.reshape(3, 2, 128, NSC, W).transpose(2, 3, 0, 1, 4)
        ).astype(ml_dtypes.float8_e4m3)
        wqt = Wq[hs, :].T * 32.0
        wkt = Wk[hs, :].T * 32.0
        in_maps.append({
            "xt": xtw,
            "xt8": x8,
            "wq0": pack8(wqt[:, 0:128]),
            "wk0": pack8(wkt[:, 0:128]),
            "wq12": pack8(wqt[:, 128:384]),
            "wk12": pack8(wkt[:, 128:384]),
            "wv": chunked(Wv[hs, :].T, NDC),
            "wo": chunked(Wo[:, hs].T, 3),
            "mk": mk,
        })

    if PROFILE:
        _install_profile_hooks()
    nc = _get_program()
    res = run_bass_kernel_spmd(nc, in_maps, core_ids=list(range(8)),
                               trace=PROFILE, tmpdir=PROFILE_DIR)
    LAST_RESULT = res
    parts = [np.asarray(res.results[c]["y"], np.float32) for c in range(8)]
    out = np.stack([parts[2 * b] + parts[2 * b + 1] + bo for b in range(B)])
    return out.astype(np.float32)



# revision 36
# speedup vs baseline: 1.0907x; 1.0438x over previous
"""Causal multi-head attention block (B=4, S=2048, D=768, H=12, Dh=64)
distributed over 8 NeuronCores: core = (batch, head-group), each core
computes its 6 heads end-to-end plus its partial output projection;
host sums the two partials per batch and adds the bias.

Self-contained: hardcodes all shapes; no sibling imports.
"""

import ml_dtypes
import numpy as np

B, S, D = 4, 2048, 768
H, DH = 12, 64
G = 384          # channels per head group (6 heads)
NPAIR = 3        # head pairs per core
NSC = 4          # 512-wide query windows
W = 512
NST = 16         # 128-row s-tiles
NDC = 6          # 128-row D chunks

_PROGRAM = None
PROFILE = False
PROFILE_DIR = None
LAST_RESULT = None


def _split_waits(nc, max_waits=1, max_updates=1):
    """This container's walrus rejects instructions carrying more than one
    semaphore wait/update ("Too many sync wait commands").  Move excess
    waits onto NoOps inserted before the owning instruction (same engine)
    and excess updates onto NoOps inserted after."""
    import concourse.mybir as mybir

    counter = [0]

    def nop(engine, waits, updates):
        counter[0] += 1
        n = mybir.InstNoOp(name=f"wsplit_nop_{counter[0]}", ins=[], outs=[])
        n.engine = engine
        n.sync_info = mybir.SyncInfo(on_wait=waits, on_update=updates)
        return n

    for bb in nc.main_func.blocks:
        out = []
        changed = False
        for ins in bb.instructions:
            si = ins.sync_info
            waits = list(si.on_wait) if si and si.on_wait else []
            updates = list(si.on_update) if si and si.on_update else []
            pre, post = [], []
            if len(waits) > max_waits:
                keep = waits[:max_waits - 1] if max_waits > 1 else []
                rest = waits[len(keep):]
                while rest:
                    chunk, rest = rest[:max_waits], rest[max_waits:]
                    pre.append(chunk)
                waits = keep
                changed = True
            if len(updates) > max_updates:
                rest = updates[max_updates:]
                updates = updates[:max_updates]
                while rest:
                    chunk, rest = rest[:max_updates], rest[max_updates:]
                    post.append(chunk)
                changed = True
            if pre or post:
                ins.sync_info = mybir.SyncInfo(
                    on_wait=waits, on_update=updates)
            for w in pre:
                out.append(nop(ins.engine, w, []))
            out.append(ins)
            for u in post:
                out.append(nop(ins.engine, [], u))
        if changed:
            bb.instructions = out


def _install_profile_hooks():
    """Dev-only (PROFILE=True): register the NTFF profile hook that the
    agent image's antenv lacks, and stub out the artifact upload."""
    import sys
    import types

    try:
        from antenv.axon_hooks import get_axon_ntff_profile_hook  # noqa: F401
    except ImportError:
        import antenv
        from trn_agent_boot import trn_boot

        hook = trn_boot._ntff_profile_via_ctypes("/opt/axon/libaxon_pjrt.so")
        mod = types.ModuleType("antenv.axon_hooks")
        mod._hook = hook
        mod.get_axon_ntff_profile_hook = lambda: mod._hook
        mod.set_axon_ntff_profile_hook = lambda h: setattr(mod, "_hook", h)
        sys.modules["antenv.axon_hooks"] = mod
        antenv.axon_hooks = mod

    from concourse import bass_utils

    bass_utils.upload_artifacts = lambda tmpdir: "local://" + tmpdir


def _build_program():
    import concourse.bass as bass
    import concourse.mybir as mybir
    import concourse.tile as tile

    f16 = mybir.dt.float16
    f32 = mybir.dt.float32
    f8 = mybir.dt.float8e4

    nc = bass.Bass()
    xt_d = nc.declare_dram_parameter("xt", [128, NST, NDC, 128], f16, isOutput=False)
    xt8_d = nc.declare_dram_parameter("xt8", [128, NSC, 3, 2, W], f8, isOutput=False)
    wq0_d = nc.declare_dram_parameter("wq0", [128, 3, 2, 128], f8, isOutput=False)
    wk0_d = nc.declare_dram_parameter("wk0", [128, 3, 2, 128], f8, isOutput=False)
    wq12_d = nc.declare_dram_parameter("wq12", [128, 3, 2, 256], f8, isOutput=False)
    wk12_d = nc.declare_dram_parameter("wk12", [128, 3, 2, 256], f8, isOutput=False)
    wv_d = nc.declare_dram_parameter("wv", [128, NDC, G], f16, isOutput=False)
    wo_d = nc.declare_dram_parameter("wo", [128, 3, D], f16, isOutput=False)
    mk_d = nc.declare_dram_parameter("mk", [128, 128], f16, isOutput=False)
    y_d = nc.declare_dram_parameter("y", [S, D], f16, isOutput=True)

    with tile.TileContext(nc) as tc:
        with (
            tc.tile_pool(name="const", bufs=1) as const,
            tc.tile_pool(name="work", bufs=3) as work,
            tc.tile_pool(name="outp", bufs=3) as outp,
            tc.tile_pool(name="ps", bufs=2, space="PSUM") as ps,
        ):
            # ---- persistent SBUF tiles ----
            # consolidated [128, chunk, cols] layouts: one DMA per tensor
            # (or per xt column-window) -- each dma_start trigger costs
            # ~600ns on its issuing engine and ~us-scale queue overhead,
            # so fewer+bigger transfers shorten the startup critically.
            xtb = const.tile([128, NST, NDC, 128], f16, name="xtb", tag="xtb")
            xt8b = const.tile([128, NSC, 3, 2, W], f8, name="xt8b", tag="xt8b")
            wq0b = const.tile([128, 3, 2, 128], f8, name="wq0b", tag="wq0b")
            wk0b = const.tile([128, 3, 2, 128], f8, name="wk0b", tag="wk0b")
            wq12b = const.tile([128, 3, 2, 256], f8, name="wq12b", tag="wq12b")
            wk12b = const.tile([128, 3, 2, 256], f8, name="wk12b", tag="wk12b")
            wvb = const.tile([128, NDC, G], f16, name="wvb", tag="wvb")
            wob = const.tile([128, 3, D], f16, name="wob", tag="wob")

            def xv(dc, st):  # [128, 128] x slice: key block st, d-chunk dc
                return xtb[:, st, dc, :]
            wv = [wvb[:, i, :] for i in range(NDC)]
            wo = [wob[:, i, :] for i in range(3)]

            def wqs(c, pair):  # [128, 2, 128] DoubleRow Wq slice
                if pair == 0:
                    return wq0b[:, c, :, :]
                return wq12b[:, c, :, 128 * (pair - 1):128 * pair]

            def wks(c, pair):
                if pair == 0:
                    return wk0b[:, c, :, :]
                return wk12b[:, c, :, 128 * (pair - 1):128 * pair]

            def x8s(c, sc):  # [128, 2, 512] DoubleRow x slice for window sc
                return xt8b[:, sc, c, :, :]
            # q/k stored fp8e4 in DoubleRow [part, ktile, col] layout:
            # ktile 0 carries the 64 dh values (heads at partitions 0:64 /
            # 64:128), ktile 1 is zero -- the matmul still runs at the fp8
            # double-pump rate (0.5 cyc/col), halving the scores cost.
            qt = [const.tile([128, 2, S], f8, name=f"qt{p}", tag=f"qt{p}")
                  for p in range(NPAIR)]
            kt = [const.tile([128, 2, S], f8, name=f"kt{p}", tag=f"kt{p}")
                  for p in range(NPAIR)]
            # vt slot (pair, h) = [v_h (64) | ones (64)]: the ones half makes
            # the pv matmul emit the softmax denominator (64-replicated) into
            # psum partitions 64:128 for free -- no separate dn matmuls.
            vt = [const.tile([128, NPAIR, 2, 128], f16, name=f"vt{t}",
                             tag=f"vt{t}")
                  for t in range(NST)]
            gt = [const.tile([128, S], f16, name=f"gt{p}", tag=f"gt{p}")
                  for p in range(NPAIR)]
            mk = const.tile([128, 128], f16, name="mk", tag="mk")

            # ---- input DMAs on the two HW DGE queues (gpsimd SW DGE only
            # for late-needed tensors).  Queue order matches need order:
            # scalar queue carries only the 4 weight tensors so wq0/wk0
            # land first; sync streams x windows so xt[0:4] lands by ~17us
            # (v-projections) instead of queueing behind the weights. ----
            nc.scalar.dma_start(out=wq0b, in_=wq0_d[:, :, :, :])
            nc.scalar.dma_start(out=wk0b, in_=wk0_d[:, :, :, :])
            nc.scalar.dma_start(out=wq12b, in_=wq12_d[:, :, :, :])
            nc.scalar.dma_start(out=wk12b, in_=wk12_d[:, :, :, :])
            nc.sync.dma_start(out=xt8b[:, 0, :, :, :], in_=xt8_d[:, 0, :, :, :])
            nc.sync.dma_start(out=mk, in_=mk_d[:, :])
            nc.sync.dma_start(out=xtb[:, 0:2, :, :], in_=xt_d[:, 0:2, :, :])
            nc.sync.dma_start(out=xtb[:, 2:4, :, :], in_=xt_d[:, 2:4, :, :])
            nc.sync.dma_start(out=xt8b[:, 1, :, :, :], in_=xt8_d[:, 1, :, :, :])
            nc.sync.dma_start(out=xtb[:, 4:8, :, :], in_=xt_d[:, 4:8, :, :])
            nc.sync.dma_start(out=xt8b[:, 2, :, :, :], in_=xt8_d[:, 2, :, :, :])
            nc.sync.dma_start(out=xtb[:, 8:12, :, :], in_=xt_d[:, 8:12, :, :])
            nc.gpsimd.dma_start(out=wvb, in_=wv_d[:, :, :])
            nc.gpsimd.dma_start(out=xtb[:, 12:16, :, :], in_=xt_d[:, 12:16, :, :])
            nc.gpsimd.dma_start(out=xt8b[:, 3, :, :, :], in_=xt8_d[:, 3, :, :, :])
            nc.sync.dma_start(out=wob, in_=wo_d[:, :, :])
            # memsets ordered by first use: pair-0 q/k zero halves gate the
            # first scores (~16us); vt[0:4] ones gate the first pv (~19us);
            # later vt / pairs 1-2 follow.  vt[0:4] ones go on DVE (quick,
            # lands before the first v-copy); the rest on gpsimd.
            nc.gpsimd.memset(qt[0][:, 1, :], 0.0)
            nc.gpsimd.memset(kt[0][:, 1, :], 0.0)
            for t in range(4):
                nc.vector.memset(vt[t][:, :, :, 64:128], 1.0)
            for t in range(4, NST):
                nc.gpsimd.memset(vt[t][:, :, :, 64:128], 1.0)
            for p in range(1, NPAIR):
                nc.gpsimd.memset(qt[p][:, 1, :], 0.0)
                nc.gpsimd.memset(kt[p][:, 1, :], 0.0)

            def act_copy(out, in_):
                # ScalarE Copy ('copy' is in every act table set => no
                # table swaps); keeps PSUM->SBUF casts off the DVE queue
                # where they'd sit behind window-end reciprocals.
                nc.scalar.activation(
                    out=out, in_=in_,
                    func=mybir.ActivationFunctionType.Copy, scale=1.0)

            def dve_copy(out, in_):
                nc.vector.tensor_copy(out=out, in_=in_)

            # ---- filler closures: projections and out-projections sliced
            # into single-matmul steps so they pad PE gaps between the
            # exp-paced attention matmuls without ever delaying a scores
            # matmul by more than one small filler. ----
            import collections
            NCH_PAIR = {0: 1, 1: 1, 2: 4}  # norm column chunks per pair:
            # pair 2's gt gates the out-projections, so normalize it in
            # 128-col chunks that unlock one outproj each
            GQ = collections.deque()   # (key, fn, is_last_of_key)
            done_keys = set()
            queued_keys = set()

            def push(key, fns):
                queued_keys.add(key)
                for i, f in enumerate(fns):
                    GQ.append((key, f, i == len(fns) - 1))

            def _run_one():
                k, f, is_last = GQ.popleft()
                f()
                if is_last:
                    done_keys.add(k)

            def force(key):
                # run ONLY this key's queued closures (in order), leaving
                # unrelated closures queued -- a deadline must not trigger
                # a burst of someone else's work on the PE queue
                if key not in queued_keys or key in done_keys:
                    return
                skipped = []
                while key not in done_keys:
                    k, f, is_last = GQ.popleft()
                    if k == key:
                        f()
                        if is_last:
                            done_keys.add(k)
                    else:
                        skipped.append((k, f, is_last))
                for item in reversed(skipped):
                    GQ.appendleft(item)

            def pop_n(n):
                for _ in range(n):
                    if not GQ:
                        return
                    _run_one()

            def qk_closures(pair, sc):
                st_ = {}

                def mk(which, c):
                    wsel = wqs if which == "q" else wks
                    tgt = qt if which == "q" else kt

                    def f():
                        if c == 0:
                            st_[which] = ps.tile(
                                [128, W], f32, name=f"{which}p{pair}_{sc}",
                                tag="sc", bufs=3)
                        nc.tensor.matmul(
                            st_[which],
                            wsel(c, pair),
                            x8s(c, sc),
                            start=(c == 0), stop=(c == 2),
                            perf_mode=mybir.MatmulPerfMode.DoubleRow)
                        if c == 2:
                            dve_copy(
                                tgt[pair][:, 0, W * sc:W * (sc + 1)],
                                st_[which])
                    return f
                return [mk(w, c) for w in ("q", "k") for c in range(3)]

            def v_closures(st):
                st_ = {}

                def mk(dc):
                    def f():
                        if dc == 0:
                            st_["vp"] = ps.tile(
                                [128, G], f32, name=f"vp{st}",
                                tag="sc", bufs=3)
                        nc.tensor.matmul(
                            st_["vp"],
                            xv(dc, st),
                            wv[dc],
                            start=(dc == 0), stop=(dc == NDC - 1))
                        if dc == NDC - 1:
                            dve_copy(
                                vt[st][:, :, :, 0:64],
                                st_["vp"].rearrange(
                                    "p (a b c) -> p a b c", a=NPAIR, b=2))
                    return f
                return [mk(dc) for dc in range(NDC)]

            def op_closures(st):
                st_ = {}

                def mk(half, cc):
                    tag = "sc"
                    lo, hi = (0, G) if half == 0 else (G, D)

                    def f():
                        if half == 0 and cc == 0:
                            for p in range(NPAIR):  # gt rows for this st
                                nch = NCH_PAIR[p]
                                force(("norm", p, st // 4,
                                       (st % 4) * nch // 4))
                        if cc == 0:
                            st_[half] = ps.tile(
                                [128, G], f32, name=f"o{half}_{st}",
                                tag=tag, bufs=3)
                        nc.tensor.matmul(
                            st_[half],
                            gt[cc][:, 128 * st:128 * (st + 1)],
                            wo[cc][:, lo:hi],
                            start=(cc == 0), stop=(cc == 2))
                        if cc == 2:
                            if half == 0:
                                st_["ob"] = outp.tile(
                                    [128, D], f16, name=f"ob{st}",
                                    tag="ob", bufs=4)
                            dve_copy(st_["ob"][:, lo:hi], st_[half])
                            if half == 1:
                                nc.sync.dma_start(
                                    out=y_d[128 * st:128 * (st + 1), :],
                                    in_=st_["ob"])
                    return f
                return [mk(h, cc) for h in (0, 1) for cc in range(3)]

            class PairAttention:
                """Per-pair attention with a software pipeline that crosses
                window boundaries: pv/dn of group g are issued after the
                scores of group g+1 (even when g+1 is in the next query
                window), so neither the PE nor ACT drains at boundaries."""

                DEPTH = 3

                def __init__(self, pair):
                    self.pair = pair
                    self.NCHUNK = NCH_PAIR[pair]
                    self.pending = []

                def scores_exp(self, sc, jb):
                    pair = self.pair
                    col0 = max(0, 128 * jb - W * sc)
                    diag = jb >= 4 * sc
                    sct = ps.tile([128, 1024], f32, name=f"sc{pair}_{sc}_{jb}",
                                  tag="sc", bufs=3)
                    nc.tensor.matmul(
                        sct[:, col0:W],
                        kt[pair][0:64, :, 128 * jb:128 * (jb + 1)],
                        qt[pair][0:64, :, W * sc + col0:W * (sc + 1)],
                        start=True, stop=True,
                        perf_mode=mybir.MatmulPerfMode.DoubleRow)
                    nc.tensor.matmul(
                        sct[:, W:2 * W - col0],
                        kt[pair][64:128, :, 128 * jb:128 * (jb + 1)],
                        qt[pair][64:128, :, W * sc + col0:W * (sc + 1)],
                        start=True, stop=True,
                        perf_mode=mybir.MatmulPerfMode.DoubleRow)
                    ex = work.tile([128, 1024], f16, name=f"ex{pair}_{sc}_{jb}",
                                   tag="exp", bufs=10)
                    nc.scalar.activation(
                        out=ex[:, col0:2 * W - col0],
                        in_=sct[:, col0:2 * W - col0],
                        func=mybir.ActivationFunctionType.Exp,
                        scale=0.125 / 1024.0)
                    if diag:  # zero the j>i triangle of the diagonal block
                        # on Pool (gpsimd): keeps the jb-critical mask off the
                        # DVE queue, where the window-end reciprocal (3.3us)
                        # would delay it and starve the PE
                        nc.gpsimd.tensor_mul(
                            ex[:, col0:col0 + 128], ex[:, col0:col0 + 128], mk)
                        nc.gpsimd.tensor_mul(
                            ex[:, W:W + 128], ex[:, W:W + 128], mk)
                    return ex

                def pv_dn(self, state):
                    pair = self.pair
                    pva, pvb, sc, jb, ex = state
                    force(("v", jb))
                    col0 = max(0, 128 * jb - W * sc)
                    first, last = (jb == 0), (jb == 4 * sc + 3)
                    # [v_h | ones] stationary: rows 0:64 = attn@v, rows
                    # 64:128 = softmax denominator (64-replicated), one
                    # matmul per head instead of pv+dn pairs.
                    nc.tensor.matmul(
                        pva[:, col0:W],
                        vt[jb][:, pair, 0, :],
                        ex[:, col0:W],
                        start=first, stop=last)
                    nc.tensor.matmul(
                        pvb[:, col0:W],
                        vt[jb][:, pair, 1, :],
                        ex[:, W:2 * W - col0],
                        start=first, stop=last)
                    if last:  # window complete: drain psum into sbuf fp16
                        # with 4 quick casts (frees pva/pvb for the next
                        # window fast).  The slow reciprocal+mul then runs
                        # from sbuf as deferred filler closures, spread
                        # across the next window's DVE queue so it never
                        # delays the next window's psum-release casts.
                        pvh = work.tile([128, W], f16, name=f"ph{pair}_{sc}",
                                        tag="pvh", bufs=3)
                        dnh = work.tile([128, W], f16, name=f"dh{pair}_{sc}",
                                        tag="dnh", bufs=3)
                        rch = work.tile([128, W], f16, name=f"rh{pair}_{sc}",
                                        tag="rch", bufs=3)
                        dve_copy(pvh[0:64, :], pva[0:64, :])
                        dve_copy(dnh[0:64, :], pva[64:128, :])
                        dve_copy(pvh[64:128, :], pvb[0:64, :])
                        dve_copy(dnh[64:128, :], pvb[64:128, :])
                        nch = self.NCHUNK
                        cw = W // nch

                        def chunk(h):
                            def f():
                                a, b = cw * h, cw * (h + 1)
                                with nc.allow_low_precision(
                                        reason="softmax denom fp16 recip"):
                                    nc.vector.reciprocal(
                                        out=rch[:, a:b], in_=dnh[:, a:b])
                                nc.vector.tensor_mul(
                                    gt[pair][:, W * sc + a:W * sc + b],
                                    pvh[:, a:b], rch[:, a:b])
                            return f
                        for h in range(nch):  # per-chunk keys so outproj st
                            # forces only the 128 gt columns it reads
                            push(("norm", pair, sc, h), [chunk(h)])

                def window(self, sc, pace):
                    pair = self.pair
                    force(("qk", pair, sc))
                    pva = ps.tile([128, W], f32, name=f"pa{pair}_{sc}",
                                  tag="apv", bufs=1)
                    pvb = ps.tile([128, W], f32, name=f"pb{pair}_{sc}",
                                  tag="adn", bufs=1)
                    for jb in range(4 * sc + 4):
                        ex = self.scores_exp(sc, jb)
                        while len(self.pending) >= self.DEPTH:
                            self.pv_dn(self.pending.pop(0))
                        self.pending.append((pva, pvb, sc, jb, ex))
                        pop_n(pace)

                def flush(self):
                    while self.pending:
                        self.pv_dn(self.pending.pop(0))

            # ---- orchestration ----
            # All projection work rides the global filler queue in deadline
            # order; deadline `force`s guarantee correctness, per-jb pop_n
            # paces the queue so the exp stream is never starved by a big
            # filler burst.
            push(("qk", 0, 0), qk_closures(0, 0))
            for st in range(4):
                push(("v", st), v_closures(st))
            force(("qk", 0, 0))
            push(("qk", 0, 1), qk_closures(0, 1))
            for st in range(4, 8):
                push(("v", st), v_closures(st))
            push(("qk", 0, 2), qk_closures(0, 2))
            for st in range(8, 12):
                push(("v", st), v_closures(st))
            push(("qk", 0, 3), qk_closures(0, 3))
            for st in range(12, NST):
                push(("v", st), v_closures(st))
            for s in range(NSC):
                push(("qk", 1, s), qk_closures(1, s))
            for s in range(NSC):
                push(("qk", 2, s), qk_closures(2, s))

            pa0 = PairAttention(0)
            for sc in range(NSC):
                pa0.window(sc, pace=3)
            pa0.flush()
            pa1 = PairAttention(1)
            for sc in range(NSC):
                pa1.window(sc, pace=1)
            pa1.flush()
            pa2 = PairAttention(2)
            for sc in range(NSC):
                pa2.window(sc, pace=2)
                # flushing per pa2 window issues the normalize chain early,
                # unlocking this window's four out-projections
                pa2.flush()
                for st in range(4 * sc, 4 * sc + 4):
                    push(("op", st), op_closures(st))
            while GQ:
                pop_n(4)

    _split_waits(nc)
    return nc


def _get_program():
    global _PROGRAM
    if _PROGRAM is None:
        _PROGRAM = _build_program()
    return _PROGRAM


def kernel(x, Wq, Wk, Wv, Wo, bo):
    global LAST_RESULT
    from concourse.bass_utils import run_bass_kernel_spmd

    x = np.asarray(x, np.float32)
    Wq = np.asarray(Wq, np.float32)
    Wk = np.asarray(Wk, np.float32)
    Wv = np.asarray(Wv, np.float32)
    Wo = np.asarray(Wo, np.float32)
    bo = np.asarray(bo, np.float32)

    tri = np.tril(np.ones((128, 128), np.float32)).T  # 1 where j<=i
    mk = tri.astype(np.float16)

    in_maps = []
    for c in range(8):
        b, g = divmod(c, 2)
        hs = slice(G * g, G * (g + 1))
        def chunked(a, n):  # [n*128, M] -> [128, n, M]
            m = a.shape[1]
            return np.ascontiguousarray(
                a.reshape(n, 128, m).transpose(1, 0, 2)).astype(np.float16)

        xtw = np.ascontiguousarray(
            x[b].T.reshape(NDC, 128, NST, 128).transpose(1, 2, 0, 3)
        ).astype(np.float16)
        def pack8(a):  # [768, M] -> [128, 3, 2, M] e4m3 (d = 256c+128t+p)
            m = a.shape[1]
            return np.ascontiguousarray(
                a.reshape(3, 2, 128, m).transpose(2, 0, 1, 3)
            ).astype(ml_dtypes.float8_e4m3)

        x8 = np.ascontiguousarray(
            x[b].T.reshape(3, 2, 128, NSC, W).transpose(2, 3, 0, 1, 4)
        ).astype(ml_dtypes.float8_e4m3)
        wqt = Wq[hs, :].T * 32.0
        wkt = Wk[hs, :].T * 32.0
        in_maps.append({
            "xt": xtw,
            "xt8": x8,
            "wq0": pack8(wqt[:, 0:128]),
            "wk0": pack8(wkt[:, 0:128]),
            "wq12": pack8(wqt[:, 128:384]),
            "wk12": pack8(wkt[:, 128:384]),
            "wv": chunked(Wv[hs, :].T, NDC),
            "wo": chunked(Wo[:, hs].T, 3),
            "mk": mk,
        })

    if PROFILE:
        _install_profile_hooks()
    nc = _get_program()
    res = run_bass_kernel_spmd(nc, in_maps, core_ids=list(range(8)),
                               trace=PROFILE, tmpdir=PROFILE_DIR)
    LAST_RESULT = res
    parts = [np.asarray(res.results[c]["y"], np.float32) for c in range(8)]
    out = np.stack([parts[2 * b] + parts[2 * b + 1] + bo for b in range(B)])
    return out.astype(np.float32)



# revision 37
# speedup vs baseline: 1.1004x; 1.0089x over previous
"""Causal multi-head attention block (B=4, S=2048, D=768, H=12, Dh=64)
distributed over 8 NeuronCores: core = (batch, head-group), each core
computes its 6 heads end-to-end plus its partial output projection;
host sums the two partials per batch and adds the bias.

Self-contained: hardcodes all shapes; no sibling imports.
"""

import ml_dtypes
import numpy as np

B, S, D = 4, 2048, 768
H, DH = 12, 64
G = 384          # channels per head group (6 heads)
NPAIR = 3        # head pairs per core
NSC = 4          # 512-wide query windows
W = 512
NST = 16         # 128-row s-tiles
NDC = 6          # 128-row D chunks

_PROGRAM = None
PROFILE = False
PROFILE_DIR = None
LAST_RESULT = None


def _split_waits(nc, max_waits=1, max_updates=1):
    """This container's walrus rejects instructions carrying more than one
    semaphore wait/update ("Too many sync wait commands").  Move excess
    waits onto NoOps inserted before the owning instruction (same engine)
    and excess updates onto NoOps inserted after."""
    import concourse.mybir as mybir

    counter = [0]

    def nop(engine, waits, updates):
        counter[0] += 1
        n = mybir.InstNoOp(name=f"wsplit_nop_{counter[0]}", ins=[], outs=[])
        n.engine = engine
        n.sync_info = mybir.SyncInfo(on_wait=waits, on_update=updates)
        return n

    for bb in nc.main_func.blocks:
        out = []
        changed = False
        for ins in bb.instructions:
            si = ins.sync_info
            waits = list(si.on_wait) if si and si.on_wait else []
            updates = list(si.on_update) if si and si.on_update else []
            pre, post = [], []
            if len(waits) > max_waits:
                keep = waits[:max_waits - 1] if max_waits > 1 else []
                rest = waits[len(keep):]
                while rest:
                    chunk, rest = rest[:max_waits], rest[max_waits:]
                    pre.append(chunk)
                waits = keep
                changed = True
            if len(updates) > max_updates:
                rest = updates[max_updates:]
                updates = updates[:max_updates]
                while rest:
                    chunk, rest = rest[:max_updates], rest[max_updates:]
                    post.append(chunk)
                changed = True
            if pre or post:
                ins.sync_info = mybir.SyncInfo(
                    on_wait=waits, on_update=updates)
            for w in pre:
                out.append(nop(ins.engine, w, []))
            out.append(ins)
            for u in post:
                out.append(nop(ins.engine, [], u))
        if changed:
            bb.instructions = out


def _install_profile_hooks():
    """Dev-only (PROFILE=True): register the NTFF profile hook that the
    agent image's antenv lacks, and stub out the artifact upload."""
    import sys
    import types

    try:
        from antenv.axon_hooks import get_axon_ntff_profile_hook  # noqa: F401
    except ImportError:
        import antenv
        from trn_agent_boot import trn_boot

        hook = trn_boot._ntff_profile_via_ctypes("/opt/axon/libaxon_pjrt.so")
        mod = types.ModuleType("antenv.axon_hooks")
        mod._hook = hook
        mod.get_axon_ntff_profile_hook = lambda: mod._hook
        mod.set_axon_ntff_profile_hook = lambda h: setattr(mod, "_hook", h)
        sys.modules["antenv.axon_hooks"] = mod
        antenv.axon_hooks = mod

    from concourse import bass_utils

    bass_utils.upload_artifacts = lambda tmpdir: "local://" + tmpdir


def _build_program():
    import concourse.bass as bass
    import concourse.mybir as mybir
    import concourse.tile as tile

    f16 = mybir.dt.float16
    f32 = mybir.dt.float32
    f8 = mybir.dt.float8e4

    nc = bass.Bass()
    xt_d = nc.declare_dram_parameter("xt", [128, NST, NDC, 128], f16, isOutput=False)
    xt8_d = nc.declare_dram_parameter("xt8", [128, NSC, 3, 2, W], f8, isOutput=False)
    wq0_d = nc.declare_dram_parameter("wq0", [128, 3, 2, 128], f8, isOutput=False)
    wk0_d = nc.declare_dram_parameter("wk0", [128, 3, 2, 128], f8, isOutput=False)
    wq12_d = nc.declare_dram_parameter("wq12", [128, 3, 2, 256], f8, isOutput=False)
    wk12_d = nc.declare_dram_parameter("wk12", [128, 3, 2, 256], f8, isOutput=False)
    wv_d = nc.declare_dram_parameter("wv", [128, NDC, G], f16, isOutput=False)
    wo_d = nc.declare_dram_parameter("wo", [128, 3, D], f16, isOutput=False)
    mk_d = nc.declare_dram_parameter("mk", [128, 128], f16, isOutput=False)
    y_d = nc.declare_dram_parameter("y", [S, D], f16, isOutput=True)

    with tile.TileContext(nc) as tc:
        with (
            tc.tile_pool(name="const", bufs=1) as const,
            tc.tile_pool(name="work", bufs=3) as work,
            tc.tile_pool(name="outp", bufs=3) as outp,
            tc.tile_pool(name="ps", bufs=2, space="PSUM") as ps,
        ):
            # ---- persistent SBUF tiles ----
            # consolidated [128, chunk, cols] layouts: one DMA per tensor
            # (or per xt column-window) -- each dma_start trigger costs
            # ~600ns on its issuing engine and ~us-scale queue overhead,
            # so fewer+bigger transfers shorten the startup critically.
            xtb = const.tile([128, NST, NDC, 128], f16, name="xtb", tag="xtb")
            xt8b = const.tile([128, NSC, 3, 2, W], f8, name="xt8b", tag="xt8b")
            wq0b = const.tile([128, 3, 2, 128], f8, name="wq0b", tag="wq0b")
            wk0b = const.tile([128, 3, 2, 128], f8, name="wk0b", tag="wk0b")
            wq12b = const.tile([128, 3, 2, 256], f8, name="wq12b", tag="wq12b")
            wk12b = const.tile([128, 3, 2, 256], f8, name="wk12b", tag="wk12b")
            wvb = const.tile([128, NDC, G], f16, name="wvb", tag="wvb")
            wob = const.tile([128, 3, D], f16, name="wob", tag="wob")

            def xv(dc, st):  # [128, 128] x slice: key block st, d-chunk dc
                return xtb[:, st, dc, :]
            wv = [wvb[:, i, :] for i in range(NDC)]
            wo = [wob[:, i, :] for i in range(3)]

            def wqs(c, pair):  # [128, 2, 128] DoubleRow Wq slice
                if pair == 0:
                    return wq0b[:, c, :, :]
                return wq12b[:, c, :, 128 * (pair - 1):128 * pair]

            def wks(c, pair):
                if pair == 0:
                    return wk0b[:, c, :, :]
                return wk12b[:, c, :, 128 * (pair - 1):128 * pair]

            def x8s(c, sc):  # [128, 2, 512] DoubleRow x slice for window sc
                return xt8b[:, sc, c, :, :]
            # q/k stored fp8e4 in DoubleRow [part, ktile, col] layout:
            # ktile 0 carries the 64 dh values (heads at partitions 0:64 /
            # 64:128), ktile 1 is zero -- the matmul still runs at the fp8
            # double-pump rate (0.5 cyc/col), halving the scores cost.
            qt = [const.tile([128, 2, S], f8, name=f"qt{p}", tag=f"qt{p}")
                  for p in range(NPAIR)]
            kt = [const.tile([128, 2, S], f8, name=f"kt{p}", tag=f"kt{p}")
                  for p in range(NPAIR)]
            # vt slot (pair, h) = [v_h (64) | ones (64)]: the ones half makes
            # the pv matmul emit the softmax denominator (64-replicated) into
            # psum partitions 64:128 for free -- no separate dn matmuls.
            vt = [const.tile([128, NPAIR, 2, 128], f16, name=f"vt{t}",
                             tag=f"vt{t}")
                  for t in range(NST)]
            gt = [const.tile([128, S], f16, name=f"gt{p}", tag=f"gt{p}")
                  for p in range(NPAIR)]
            mk = const.tile([128, 128], f16, name="mk", tag="mk")

            # ---- input DMAs on the two HW DGE queues (gpsimd SW DGE only
            # for late-needed tensors).  Queue order matches need order:
            # scalar queue carries only the 4 weight tensors so wq0/wk0
            # land first; sync streams x windows so xt[0:4] lands by ~17us
            # (v-projections) instead of queueing behind the weights. ----
            nc.scalar.dma_start(out=wq0b, in_=wq0_d[:, :, :, :])
            nc.scalar.dma_start(out=wk0b, in_=wk0_d[:, :, :, :])
            nc.scalar.dma_start(out=wq12b, in_=wq12_d[:, :, :, :])
            nc.scalar.dma_start(out=wk12b, in_=wk12_d[:, :, :, :])
            nc.sync.dma_start(out=xt8b[:, 0, :, :, :], in_=xt8_d[:, 0, :, :, :])
            nc.sync.dma_start(out=mk, in_=mk_d[:, :])
            nc.sync.dma_start(out=xtb[:, 0:2, :, :], in_=xt_d[:, 0:2, :, :])
            nc.sync.dma_start(out=xtb[:, 2:4, :, :], in_=xt_d[:, 2:4, :, :])
            nc.sync.dma_start(out=xt8b[:, 1, :, :, :], in_=xt8_d[:, 1, :, :, :])
            nc.sync.dma_start(out=xtb[:, 4:8, :, :], in_=xt_d[:, 4:8, :, :])
            nc.sync.dma_start(out=xt8b[:, 2, :, :, :], in_=xt8_d[:, 2, :, :, :])
            nc.sync.dma_start(out=xtb[:, 8:12, :, :], in_=xt_d[:, 8:12, :, :])
            nc.gpsimd.dma_start(out=wvb, in_=wv_d[:, :, :])
            nc.gpsimd.dma_start(out=xtb[:, 12:16, :, :], in_=xt_d[:, 12:16, :, :])
            nc.gpsimd.dma_start(out=xt8b[:, 3, :, :, :], in_=xt8_d[:, 3, :, :, :])
            nc.sync.dma_start(out=wob, in_=wo_d[:, :, :])
            # memsets ordered by first use: pair-0 q/k zero halves gate the
            # first scores (~16us); vt[0:4] ones gate the first pv (~19us);
            # later vt / pairs 1-2 follow.  vt[0:4] ones go on DVE (quick,
            # lands before the first v-copy); the rest on gpsimd.
            nc.gpsimd.memset(qt[0][:, 1, :], 0.0)
            nc.gpsimd.memset(kt[0][:, 1, :], 0.0)
            for t in range(4):
                nc.vector.memset(vt[t][:, :, :, 64:128], 1.0)
            for t in range(4, NST):
                nc.gpsimd.memset(vt[t][:, :, :, 64:128], 1.0)
            for p in range(1, NPAIR):
                nc.gpsimd.memset(qt[p][:, 1, :], 0.0)
                nc.gpsimd.memset(kt[p][:, 1, :], 0.0)

            def act_copy(out, in_):
                # ScalarE Copy ('copy' is in every act table set => no
                # table swaps); keeps PSUM->SBUF casts off the DVE queue
                # where they'd sit behind window-end reciprocals.
                nc.scalar.activation(
                    out=out, in_=in_,
                    func=mybir.ActivationFunctionType.Copy, scale=1.0)

            def dve_copy(out, in_):
                nc.vector.tensor_copy(out=out, in_=in_)

            # ---- filler closures: projections and out-projections sliced
            # into single-matmul steps so they pad PE gaps between the
            # exp-paced attention matmuls without ever delaying a scores
            # matmul by more than one small filler. ----
            import collections
            NCH_PAIR = {0: 2, 1: 2, 2: 4}  # norm column chunks per pair:
            # pair 2's gt gates the out-projections, so normalize it in
            # 128-col chunks that unlock one outproj each
            GQ = collections.deque()   # (key, fn, is_last_of_key)
            done_keys = set()
            queued_keys = set()

            def push(key, fns):
                queued_keys.add(key)
                for i, f in enumerate(fns):
                    GQ.append((key, f, i == len(fns) - 1))

            def _run_one():
                k, f, is_last = GQ.popleft()
                f()
                if is_last:
                    done_keys.add(k)

            def force(key):
                # run ONLY this key's queued closures (in order), leaving
                # unrelated closures queued -- a deadline must not trigger
                # a burst of someone else's work on the PE queue
                if key not in queued_keys or key in done_keys:
                    return
                skipped = []
                while key not in done_keys:
                    k, f, is_last = GQ.popleft()
                    if k == key:
                        f()
                        if is_last:
                            done_keys.add(k)
                    else:
                        skipped.append((k, f, is_last))
                for item in reversed(skipped):
                    GQ.appendleft(item)

            def pop_n(n):
                for _ in range(n):
                    if not GQ:
                        return
                    _run_one()

            def qk_closures(pair, sc):
                st_ = {}

                def mk(which, c):
                    wsel = wqs if which == "q" else wks
                    tgt = qt if which == "q" else kt

                    def f():
                        if c == 0:
                            st_[which] = ps.tile(
                                [128, W], f32, name=f"{which}p{pair}_{sc}",
                                tag="sc", bufs=3)
                        nc.tensor.matmul(
                            st_[which],
                            wsel(c, pair),
                            x8s(c, sc),
                            start=(c == 0), stop=(c == 2),
                            perf_mode=mybir.MatmulPerfMode.DoubleRow)
                        if c == 2:
                            dve_copy(
                                tgt[pair][:, 0, W * sc:W * (sc + 1)],
                                st_[which])
                    return f
                return [mk(w, c) for w in ("q", "k") for c in range(3)]

            def v_closures(st):
                st_ = {}

                def mk(dc):
                    def f():
                        if dc == 0:
                            st_["vp"] = ps.tile(
                                [128, G], f32, name=f"vp{st}",
                                tag="sc", bufs=3)
                        nc.tensor.matmul(
                            st_["vp"],
                            xv(dc, st),
                            wv[dc],
                            start=(dc == 0), stop=(dc == NDC - 1))
                        if dc == NDC - 1:
                            dve_copy(
                                vt[st][:, :, :, 0:64],
                                st_["vp"].rearrange(
                                    "p (a b c) -> p a b c", a=NPAIR, b=2))
                    return f
                return [mk(dc) for dc in range(NDC)]

            def op_closures(st):
                st_ = {}

                def mk(half, cc):
                    tag = "sc"
                    lo, hi = (0, G) if half == 0 else (G, D)

                    def f():
                        if half == 0 and cc == 0:
                            for p in range(NPAIR):  # gt rows for this st
                                nch = NCH_PAIR[p]
                                force(("norm", p, st // 4,
                                       (st % 4) * nch // 4))
                        if cc == 0:
                            st_[half] = ps.tile(
                                [128, G], f32, name=f"o{half}_{st}",
                                tag=tag, bufs=3)
                        nc.tensor.matmul(
                            st_[half],
                            gt[cc][:, 128 * st:128 * (st + 1)],
                            wo[cc][:, lo:hi],
                            start=(cc == 0), stop=(cc == 2))
                        if cc == 2:
                            if half == 0:
                                st_["ob"] = outp.tile(
                                    [128, D], f16, name=f"ob{st}",
                                    tag="ob", bufs=4)
                            dve_copy(st_["ob"][:, lo:hi], st_[half])
                            if half == 1:
                                nc.sync.dma_start(
                                    out=y_d[128 * st:128 * (st + 1), :],
                                    in_=st_["ob"])
                    return f
                return [mk(h, cc) for h in (0, 1) for cc in range(3)]

            class PairAttention:
                """Per-pair attention with a software pipeline that crosses
                window boundaries: pv/dn of group g are issued after the
                scores of group g+1 (even when g+1 is in the next query
                window), so neither the PE nor ACT drains at boundaries."""

                DEPTH = 3

                def __init__(self, pair):
                    self.pair = pair
                    self.NCHUNK = NCH_PAIR[pair]
                    self.pending = []

                def scores_exp(self, sc, jb):
                    pair = self.pair
                    col0 = max(0, 128 * jb - W * sc)
                    diag = jb >= 4 * sc
                    sct = ps.tile([128, 1024], f32, name=f"sc{pair}_{sc}_{jb}",
                                  tag="sc", bufs=3)
                    nc.tensor.matmul(
                        sct[:, col0:W],
                        kt[pair][0:64, :, 128 * jb:128 * (jb + 1)],
                        qt[pair][0:64, :, W * sc + col0:W * (sc + 1)],
                        start=True, stop=True,
                        perf_mode=mybir.MatmulPerfMode.DoubleRow)
                    nc.tensor.matmul(
                        sct[:, W:2 * W - col0],
                        kt[pair][64:128, :, 128 * jb:128 * (jb + 1)],
                        qt[pair][64:128, :, W * sc + col0:W * (sc + 1)],
                        start=True, stop=True,
                        perf_mode=mybir.MatmulPerfMode.DoubleRow)
                    ex = work.tile([128, 1024], f16, name=f"ex{pair}_{sc}_{jb}",
                                   tag="exp", bufs=10)
                    nc.scalar.activation(
                        out=ex[:, col0:2 * W - col0],
                        in_=sct[:, col0:2 * W - col0],
                        func=mybir.ActivationFunctionType.Exp,
                        scale=0.125 / 1024.0)
                    if diag:  # zero the j>i triangle of the diagonal block
                        # on Pool (gpsimd): keeps the jb-critical mask off the
                        # DVE queue, where the window-end reciprocal (3.3us)
                        # would delay it and starve the PE
                        nc.gpsimd.tensor_mul(
                            ex[:, col0:col0 + 128], ex[:, col0:col0 + 128], mk)
                        nc.gpsimd.tensor_mul(
                            ex[:, W:W + 128], ex[:, W:W + 128], mk)
                    return ex

                def pv_dn(self, state):
                    pair = self.pair
                    pva, pvb, sc, jb, ex = state
                    force(("v", jb))
                    col0 = max(0, 128 * jb - W * sc)
                    first, last = (jb == 0), (jb == 4 * sc + 3)
                    # [v_h | ones] stationary: rows 0:64 = attn@v, rows
                    # 64:128 = softmax denominator (64-replicated), one
                    # matmul per head instead of pv+dn pairs.
                    nc.tensor.matmul(
                        pva[:, col0:W],
                        vt[jb][:, pair, 0, :],
                        ex[:, col0:W],
                        start=first, stop=last)
                    nc.tensor.matmul(
                        pvb[:, col0:W],
                        vt[jb][:, pair, 1, :],
                        ex[:, W:2 * W - col0],
                        start=first, stop=last)
                    if last:  # window complete: drain psum into sbuf fp16
                        # with 4 quick casts (frees pva/pvb for the next
                        # window fast).  The slow reciprocal+mul then runs
                        # from sbuf as deferred filler closures, spread
                        # across the next window's DVE queue so it never
                        # delays the next window's psum-release casts.
                        pvh = work.tile([128, W], f16, name=f"ph{pair}_{sc}",
                                        tag="pvh", bufs=3)
                        dnh = work.tile([128, W], f16, name=f"dh{pair}_{sc}",
                                        tag="dnh", bufs=3)
                        rch = work.tile([128, W], f16, name=f"rh{pair}_{sc}",
                                        tag="rch", bufs=3)
                        dve_copy(pvh[0:64, :], pva[0:64, :])
                        dve_copy(dnh[0:64, :], pva[64:128, :])
                        dve_copy(pvh[64:128, :], pvb[0:64, :])
                        dve_copy(dnh[64:128, :], pvb[64:128, :])
                        nch = self.NCHUNK
                        cw = W // nch

                        def chunk(h):
                            def f():
                                a, b = cw * h, cw * (h + 1)
                                with nc.allow_low_precision(
                                        reason="softmax denom fp16 recip"):
                                    nc.vector.reciprocal(
                                        out=rch[:, a:b], in_=dnh[:, a:b])
                                nc.vector.tensor_mul(
                                    gt[pair][:, W * sc + a:W * sc + b],
                                    pvh[:, a:b], rch[:, a:b])
                            return f
                        for h in range(nch):  # per-chunk keys so outproj st
                            # forces only the 128 gt columns it reads
                            push(("norm", pair, sc, h), [chunk(h)])

                def window(self, sc, pace):
                    pair = self.pair
                    force(("qk", pair, sc))
                    pva = ps.tile([128, W], f32, name=f"pa{pair}_{sc}",
                                  tag="apv", bufs=1)
                    pvb = ps.tile([128, W], f32, name=f"pb{pair}_{sc}",
                                  tag="adn", bufs=1)
                    for jb in range(4 * sc + 4):
                        ex = self.scores_exp(sc, jb)
                        while len(self.pending) >= self.DEPTH:
                            self.pv_dn(self.pending.pop(0))
                        self.pending.append((pva, pvb, sc, jb, ex))
                        pop_n(pace)

                def flush(self):
                    while self.pending:
                        self.pv_dn(self.pending.pop(0))

            # ---- orchestration ----
            # All projection work rides the global filler queue in deadline
            # order; deadline `force`s guarantee correctness, per-jb pop_n
            # paces the queue so the exp stream is never starved by a big
            # filler burst.
            push(("qk", 0, 0), qk_closures(0, 0))
            for st in range(4):
                push(("v", st), v_closures(st))
            force(("qk", 0, 0))
            push(("qk", 0, 1), qk_closures(0, 1))
            for st in range(4, 8):
                push(("v", st), v_closures(st))
            push(("qk", 0, 2), qk_closures(0, 2))
            for st in range(8, 12):
                push(("v", st), v_closures(st))
            push(("qk", 0, 3), qk_closures(0, 3))
            for st in range(12, NST):
                push(("v", st), v_closures(st))
            for s in range(NSC):
                push(("qk", 1, s), qk_closures(1, s))
            for s in range(NSC):
                push(("qk", 2, s), qk_closures(2, s))

            pa0 = PairAttention(0)
            for sc in range(NSC):
                pa0.window(sc, pace=3)
            pa0.flush()
            pa1 = PairAttention(1)
            for sc in range(NSC):
                pa1.window(sc, pace=1)
            pa1.flush()
            pa2 = PairAttention(2)
            for sc in range(NSC):
                pa2.window(sc, pace=2)
                # flushing per pa2 window issues the normalize chain early,
                # unlocking this window's four out-projections
                pa2.flush()
                for st in range(4 * sc, 4 * sc + 4):
                    push(("op", st), op_closures(st))
            while GQ:
                pop_n(4)

    _split_waits(nc)
    return nc


def _get_program():
    global _PROGRAM
    if _PROGRAM is None:
        _PROGRAM = _build_program()
    return _PROGRAM


def kernel(x, Wq, Wk, Wv, Wo, bo):
    global LAST_RESULT
    from concourse.bass_utils import run_bass_kernel_spmd

    x = np.asarray(x, np.float32)
    Wq = np.asarray(Wq, np.float32)
    Wk = np.asarray(Wk, np.float32)
    Wv = np.asarray(Wv, np.float32)
    Wo = np.asarray(Wo, np.float32)
    bo = np.asarray(bo, np.float32)

    tri = np.tril(np.ones((128, 128), np.float32)).T  # 1 where j<=i
    mk = tri.astype(np.float16)

    in_maps = []
    for c in range(8):
        b, g = divmod(c, 2)
        hs = slice(G * g, G * (g + 1))
        def chunked(a, n):  # [n*128, M] -> [128, n, M]
            m = a.shape[1]
            return np.ascontiguousarray(
                a.reshape(n, 128, m).transpose(1, 0, 2)).astype(np.float16)

        xtw = np.ascontiguousarray(
            x[b].T.reshape(NDC, 128, NST, 128).transpose(1, 2, 0, 3)
        ).astype(np.float16)
        def pack8(a):  # [768, M] -> [128, 3, 2, M] e4m3 (d = 256c+128t+p)
            m = a.shape[1]
            return np.ascontiguousarray(
                a.reshape(3, 2, 128, m).transpose(2, 0, 1, 3)
            ).astype(ml_dtypes.float8_e4m3)

        x8 = np.ascontiguousarray(
            x[b].T.reshape(3, 2, 128, NSC, W).transpose(2, 3, 0, 1, 4)
        ).astype(ml_dtypes.float8_e4m3)
        wqt = Wq[hs, :].T * 32.0
        wkt = Wk[hs, :].T * 32.0
        in_maps.append({
            "xt": xtw,
            "xt8": x8,
            "wq0": pack8(wqt[:, 0:128]),
            "wk0": pack8(wkt[:, 0:128]),
            "wq12": pack8(wqt[:, 128:384]),
            "wk12": pack8(wkt[:, 128:384]),
            "wv": chunked(Wv[hs, :].T, NDC),
            "wo": chunked(Wo[:, hs].T, 3),
            "mk": mk,
        })

    if PROFILE:
        _install_profile_hooks()
    nc = _get_program()
    res = run_bass_kernel_spmd(nc, in_maps, core_ids=list(range(8)),
                               trace=PROFILE, tmpdir=PROFILE_DIR)
    LAST_RESULT = res
    parts = [np.asarray(res.results[c]["y"], np.float32) for c in range(8)]
    out = np.stack([parts[2 * b] + parts[2 * b + 1] + bo for b in range(B)])
    return out.astype(np.float32)

